# revision 21
# baseline (speedup 1.0000x reference)
"""Trainium2 Bass kernel for the Castro2025 RL model (T=1000, B=8192, P=1024, A=4).

Sharding: batch (sessions) split across 8 NeuronCores; per-participant params
replicated; sequential scan over T runs on-device per core.

Structure per core (1024 sessions as [128 partitions, G=8 groups]):
 - scaled serial loop on Vector: 4 ops/trial emitting the per-trial sum
   sequence m-tilde; the per-step decay product A_t is folded into
   pre-divided targets within 10-step windows (rescale at window ends).
 - bulk reconstruction: per-arm affine scan (tensor_tensor_scan) gives
   q_t for all t; softmax/log pipeline with pairwise A-reductions,
   activations on the Scalar engine, mask work on GpSimd.
Self-contained: includes the harness patches it needs.
"""
import sys
import types
import numpy as np
import concourse.bass as bass
import concourse.bacc as bacc
import concourse.mybir as mybir
import concourse.tile as tile
from concourse.vector_clock import ScopedClock

# ---- harness patch: this walrus build rejects any instruction carrying more
# than one semaphore wait; split extras onto single-wait NOPs beforehand. ----
MAX_WAITS = 1
if not getattr(tile, "_waitsplit_patched", False):
    _orig_postorder = tile.postorder_instruction_blocks

    def _split_waits_postorder(ordered, start_bb, postordered):
        out = _orig_postorder(ordered, start_bb, postordered)
        for bb_name, insts in postordered.items():
            new_list = []
            for inst in insts:
                si = inst.sync_info
                if si is not None and si.on_wait and len(si.on_wait) > MAX_WAITS \
                        and inst.engine != mybir.EngineType.Unassigned \
                        and not isinstance(inst, tile.BassTileLoopBlock):
                    waits = list(si.on_wait)
                    keep = waits[-MAX_WAITS:]
                    extra = waits[:-MAX_WAITS]
                    for wi, w in enumerate(extra):
                        nop = mybir.InstNoOp(
                            name=f"I-waitsplit-{id(inst)}-{len(new_list)}-{wi}",
                            engine=inst.engine,
                            sync_info=mybir.SyncInfo(on_wait=[w], on_update=[]),
                        )
                        new_list.append(nop)
                    si.on_wait = keep
                new_list.append(inst)
            insts[:] = new_list
        return out

    tile.postorder_instruction_blocks = _split_waits_postorder

    def _patched_drain_and_barrier(self, tick_clock, wait_clock):
        probe = mybir.InstNoOp(name=f"I-{self.nc.next_id()}", engine=mybir.EngineType.SP)
        wait_clock.add_sem_waits(probe, ScopedClock({None: tick_clock.global_clock}))
        waits = list(probe.sync_info.on_wait) if probe.sync_info and probe.sync_info.on_wait else []
        for w in waits:
            nop = self.nc.sync.nop(nofuse=True, hint="drain_split_wait")
            if nop.ins.sync_info is None:
                nop.ins.sync_info = mybir.SyncInfo(on_wait=[w], on_update=[])
            else:
                nop.ins.sync_info.on_wait = [w]
        self.nc.sync.drain()
        self.nc.all_engine_barrier()
        popped = self.nc._tile_sem_poison_stack.pop()
        assert popped is self._sem_poison
        self.nc.clear_and_free_semaphores(list(self.sems.allocated().values()))
        self.nc.all_engine_barrier()

    tile.TileContext._drain_and_barrier = _patched_drain_and_barrier
    tile._waitsplit_patched = True

from concourse.bass_utils import run_bass_kernel_spmd  # noqa: E402

F32 = mybir.dt.float32
F16 = mybir.dt.float16
I32 = mybir.dt.int32
U8 = mybir.dt.uint8
AX = mybir.AxisListType
OP = mybir.AluOpType
AF = mybir.ActivationFunctionType

T, B, P, A = 1000, 8192, 1024, 4
NCORE = 8
BL = B // NCORE          # 1024 sessions per core
G = BL // 128            # 8 groups
ER_D = 1.0 - 1e-3
LN_ER_D = float(np.log(ER_D))
TCH = 50                 # chunk length
NCH = T // TCH
W = 10                   # scale window length
NW = TCH // W


def bcA(ap, n=A):
    """[...] -> broadcast a new axis before the last (trial) axis."""
    s = list(ap.shape)
    k = len(s) - 1
    return ap.unsqueeze(k).broadcast_to(s[:k] + [n] + s[k:])


def bcT(ap, n):
    """append broadcast trailing axis of n."""
    s = list(ap.shape)
    return ap.unsqueeze(len(s)).broadcast_to(s + [n])


def build_nc():
    nc = bacc.Bacc()
    rew_d = nc.declare_dram_parameter("rewards", [BL, T], F32, isOutput=False)
    cho_d = nc.declare_dram_parameter("choices", [BL, T], I32, isOutput=False)
    par_d = nc.declare_dram_parameter("paramsT", [P, 13], F32, isOutput=False)
    pid_d = nc.declare_dram_parameter("pids", [128, G], I32, isOutput=False)
    out_d = nc.declare_dram_parameter("out", [128, G, A, T], F32, isOutput=True)

    with tile.TileContext(nc) as tc:
        import contextlib
        with contextlib.ExitStack() as ctx:
            _build(ctx, tc, nc, rew_d, cho_d, par_d, pid_d, out_d)
    nc.compile()
    return nc


def _build(ctx, tc, nc, rew_d, cho_d, par_d, pid_d, out_d):
    pp = ctx.enter_context(tc.tile_pool(name="persist", bufs=1))
    wp = ctx.enter_context(tc.tile_pool(name="work", bufs=2))
    wq = ctx.enter_context(tc.tile_pool(name="workq", bufs=1))
    wqs = ctx.enter_context(tc.tile_pool(name="workqs", bufs=2))

    V, PL, SC = nc.vector, nc.gpsimd, nc.scalar

    # ---------- params ----------
    pids = pp.tile([128, G], I32)
    nc.sync.dma_start(pids[:], pid_d[:])
    praw = pp.tile([128, G, 13], F32)
    for g in range(G):
        nc.gpsimd.indirect_dma_start(
            out=praw[:, g, :], out_offset=None, in_=par_d[:],
            in_offset=bass.IndirectOffsetOnAxis(ap=pids[:, g:g + 1], axis=0))
    V.tensor_scalar(praw[:], praw[:], -5.0, 5.0, op0=OP.max, op1=OP.min)

    c_one = pp.tile([128, 1], F32)
    V.memset(c_one[:], 1.0)
    c_lnd = pp.tile([128, 1], F32)
    V.memset(c_lnd[:], LN_ER_D)

    def sp(dst, src):
        SC.activation(dst, src, AF.Exp)
        SC.activation(dst, dst, AF.Ln, bias=c_one[:])

    def sg(dst, src):
        SC.activation(dst, src, AF.Sigmoid)

    def clip(ap, lo, hi):
        V.tensor_scalar(ap, ap, float(lo), float(hi), op0=OP.max, op1=OP.min)

    NPV = 20
    pv = pp.tile([128, G, NPV], F32)
    (BETA_R, LAPSE, PRIOR, AER, DECAY, AB1, AB2, PERV, SW, GAM, TEMP, BETA_P,
     A1, L4, OML, PWSW, GAMP1, QD, RD, SWX) = range(NPV)
    sp(pv[:, :, BETA_R], praw[:, :, 0]); clip(pv[:, :, BETA_R], 0.01, 20.0)
    sg(pv[:, :, LAPSE], praw[:, :, 1]); clip(pv[:, :, LAPSE], 0.01, 0.99)
    sp(pv[:, :, PRIOR], praw[:, :, 2]); clip(pv[:, :, PRIOR], 0.01, 0.99)
    sg(pv[:, :, AER], praw[:, :, 3]); clip(pv[:, :, AER], 0.01, 0.99)
    sg(pv[:, :, DECAY], praw[:, :, 4]); clip(pv[:, :, DECAY], 0.01, 0.99)
    V.tensor_copy(pv[:, :, AB1], praw[:, :, 5])
    V.tensor_copy(pv[:, :, AB2], praw[:, :, 6])
    sp(pv[:, :, PERV], praw[:, :, 7])
    V.tensor_copy(pv[:, :, SW], praw[:, :, 8])
    sp(pv[:, :, GAM], praw[:, :, 10])
    sp(pv[:, :, TEMP], praw[:, :, 11])
    V.tensor_scalar(pv[:, :, TEMP], pv[:, :, TEMP], 1e-6, None, op0=OP.add)
    clip(pv[:, :, TEMP], 1e-6, 100.0)
    sp(pv[:, :, BETA_P], praw[:, :, 12])
    rtmp = pp.tile([128, G], F32)
    V.reciprocal(rtmp[:], pv[:, :, TEMP])
    V.tensor_tensor(pv[:, :, A1], pv[:, :, BETA_R], rtmp[:], op=OP.mult)
    V.tensor_scalar(pv[:, :, L4], pv[:, :, LAPSE], 0.25, None, op0=OP.mult)
    V.tensor_scalar(pv[:, :, OML], pv[:, :, LAPSE], -1.0, 1.0, op0=OP.mult, op1=OP.add)
    V.tensor_tensor(pv[:, :, PWSW], pv[:, :, PERV], pv[:, :, SW], op=OP.subtract)
    V.tensor_scalar(pv[:, :, GAMP1], pv[:, :, GAM], 1.0, None, op0=OP.add)
    V.tensor_scalar(pv[:, :, QD], pv[:, :, DECAY], 0.25, None, op0=OP.mult)
    V.tensor_copy(pv[:, :, SWX], pv[:, :, SW])

    # dp[t] = 0.999^(t+1)
    dpow = pp.tile([128, T], F32)
    ii32 = pp.tile([128, T], I32)
    nc.gpsimd.iota(ii32[:], pattern=[[1, T]], base=0, channel_multiplier=0)
    V.tensor_copy(dpow[:], ii32[:])
    SC.activation(dpow[:], dpow[:], AF.Exp, bias=c_lnd[:], scale=c_lnd[:])

    # reset masks for per-chunk scans
    rmGA = pp.tile([128, G, A, TCH], F32)
    V.memset(rmGA[:], 1.0)
    V.memset(rmGA[:, :, :, 0:1], 0.0)
    rmG = rmGA[:, :, 0, :]
    rmH = pp.tile([128, G, A, TCH], F16)
    V.memset(rmH[:], 1.0)
    V.memset(rmH[:, :, :, 0:1], 0.0)


    # serial-loop state
    qT = pp.tile([128, G, A], F32)
    V.tensor_copy(qT[:], bcT(pv[:, :, PRIOR], A))
    uT = pp.tile([128, G], F32)

    # epilogue carries
    cumc = pp.tile([128, G, A], F32)
    V.memset(cumc[:], 0.0)
    tslc = pp.tile([128, G], F32)
    V.memset(tslc[:], 0.0)

    # ---------------- per-chunk prep (Pool + a couple of V ops) ----------------
    def prep(c):
        t0 = c * TCH
        d = {}
        cho = wp.tile([128, G, TCH + 1], I32, tag="cho")
        if c == 0:
            V.memset(cho[:, :, 0:1], -1)
            nc.sync.dma_start(cho[:, :, 1:].opt(), cho_d[:, t0:t0 + TCH].rearrange("(g p) t -> p g t", p=128))
        else:
            nc.sync.dma_start(cho[:].opt(), cho_d[:, t0 - 1:t0 + TCH].rearrange("(g p) t -> p g t", p=128))
        rew = wp.tile([128, G, TCH], F32, tag="rew")
        nc.sync.dma_start(rew[:], rew_d[:, t0:t0 + TCH].rearrange("(g p) t -> p g t", p=128))

        # one-hot masks: u8 (for copy_pred) and f32 (for scan d1 / mask mults)
        ohu = wp.tile([128, G, A, TCH + 1], U8, tag="ohu")
        ohFc = wp.tile([128, G, A, TCH], F32, tag="ohFc")
        for a in range(A):
            V.tensor_scalar(ohu[:, :, a, :], cho[:], float(a), None, op0=OP.is_equal)
        SC.activation(ohFc[:], ohu[:, :, :, 1:], AF.Copy)
        ohH = wp.tile([128, G, A, TCH], F16, tag="ohH")
        SC.activation(ohH[:], ohu[:, :, :, 1:], AF.Copy)

        # e = aer * dp_t ; At = decay*(1-e) ; w~ = e/(4(1-e)) ; Bt*m needs 0.25*decay*e
        dpc = dpow[:, t0:t0 + TCH].unsqueeze(1).broadcast_to([128, G, TCH])
        e = wp.tile([128, G, TCH], F32, tag="e")
        PL.tensor_tensor(e[:], bcT(pv[:, :, AER], TCH), dpc, op=OP.mult)
        ome = wp.tile([128, G, TCH], F32, tag="ome")
        PL.tensor_scalar(ome[:], e[:], -1.0, 1.0, op0=OP.mult, op1=OP.add)   # 1-e
        At = wp.tile([128, G, TCH], F32, tag="At")
        PL.tensor_tensor(At[:], bcT(pv[:, :, DECAY], TCH), ome[:], op=OP.mult)
        rome = wp.tile([128, G, TCH], F32, tag="rome")
        V.reciprocal_approx_fast(rome[:], ome[:])                         # 1/(1-e)
        wt = wp.tile([128, G, TCH], F32, tag="wt")
        PL.tensor_tensor(wt[:], e[:], rome[:], op=OP.mult)
        PL.tensor_scalar(wt[:], wt[:], 0.25, None, op0=OP.mult)

        # tgt = rew*(1+gam) - gam
        tgt = wp.tile([128, G, TCH], F32, tag="tgt")
        PL.tensor_tensor(tgt[:], rew[:], bcT(pv[:, :, GAMP1], TCH), op=OP.mult)
        PL.tensor_tensor(tgt[:], tgt[:], bcT(pv[:, :, GAM], TCH), op=OP.subtract)

        # windowed prefix products: Pw[w,j] = prod_{u<j} At[w,u]  (Pw[w,0]=1)
        # and rPw = 1/Pw (built from rAt = 1/At = rdecay * (1/(1-e)))
        Pw = wp.tile([128, G, NW, W], F32, tag="Pw")
        rPw = wp.tile([128, G, NW, W], F32, tag="rPw")
        rAt = wp.tile([128, G, TCH], F32, tag="rAt")
        PL.tensor_tensor(rAt[:], rome[:], bcT(rtmp2[:], TCH), op=OP.mult)
        if c < 2:
            PL.memset(Pw[:, :, :, 0:1], 1.0)
            PL.memset(rPw[:, :, :, 0:1], 1.0)
        Atw = At[:].rearrange("p g (w j) -> p g w j", j=W)
        rAtw = rAt[:].rearrange("p g (w j) -> p g w j", j=W)
        for j in range(1, W):
            PL.tensor_tensor(Pw[:, :, :, j], Pw[:, :, :, j - 1], Atw[:, :, :, j - 1], op=OP.mult)
            PL.tensor_tensor(rPw[:, :, :, j], rPw[:, :, :, j - 1], rAtw[:, :, :, j - 1], op=OP.mult)
        # Dwin[w] = Pw[w, 9] * At[w, 9]
        Dwin = wp.tile([128, G, NW], F32, tag="Dwin")
        PL.tensor_tensor(Dwin[:], Pw[:, :, :, W - 1], Atw[:, :, :, W - 1], op=OP.mult)
        # tau = tgt * rPw
        tau = wp.tile([128, G, TCH], F32, tag="tau")
        PL.tensor_tensor(tau[:], tgt[:], rPw[:].rearrange("p g w j -> p g (w j)"), op=OP.mult)

        d.update(cho=cho, rew=rew, ohu=ohu, ohFc=ohFc, ohH=ohH, e=e, At=At, wt=wt,
                 tgt=tgt, Pw=Pw, Dwin=Dwin, tau=tau)
        return d

    # 1/decay (needed for rAt) -- compute once
    rtmp2 = pp.tile([128, G], F32)
    V.reciprocal(rtmp2[:], pv[:, :, DECAY])

    # ---------------- serial loop over one chunk (Vector) ----------------
    def loop(c, d):
        mt = wp.tile([128, G, TCH], F32, tag="mt")
        d["mt"] = mt
        ohu, tau, wt, Dwin = d["ohu"], d["tau"], d["wt"], d["Dwin"]
        for ti in range(TCH):
            V.copy_predicated(qT[:], ohu[:, :, :, ti + 1],
                              bcT(tau[:, :, ti], A))
            V.tensor_reduce(mt[:, :, ti], qT[:], axis=AX.X, op=OP.add)
            V.tensor_tensor(uT[:], mt[:, :, ti], wt[:, :, ti], op=OP.mult)
            V.tensor_tensor(qT[:], qT[:], bcT(uT[:], A), op=OP.add)
            if ti % W == W - 1:
                V.tensor_tensor(qT[:], qT[:], bcT(Dwin[:, :, ti // W], A), op=OP.mult)

    # ---------------- epilogue (bulk reconstruction; Pool/Act/V) ----------------
    qsprev = [None]

    def epilogue(c, d):
        t0 = c * TCH
        ohu, ohFc = d["ohu"], d["ohFc"]
        e, At, tgt, Pw, tau = d["e"], d["At"], d["tgt"], d["Pw"], d["tau"]
        cho, rew = d["cho"], d["rew"]
        At4 = bcA(At[:])

        # m = m~ * Pw ; u1 = At*tgt ; u2 = 0.25*decay*e*m
        m = wp.tile([128, G, TCH], F32, tag="m")
        PL.tensor_tensor(m[:], d["mt"][:], Pw[:].rearrange("p g w j -> p g (w j)"), op=OP.mult)
        u1 = wp.tile([128, G, TCH], F32, tag="u1")
        PL.tensor_tensor(u1[:], At[:], tgt[:], op=OP.mult)
        u2 = wp.tile([128, G, TCH], F32, tag="u2")
        PL.tensor_tensor(u2[:], e[:], m[:], op=OP.mult)
        PL.tensor_tensor(u2[:], u2[:], bcT(pv[:, :, QD], TCH), op=OP.mult)

        # al = At*(1-oh) ; be = oh*u1 + u2   (copy + copy_pred injection)
        al = wq.tile([128, G, A, TCH], F32, tag="al")
        SC.activation(al[:], At4, AF.Copy)
        V.copy_predicated(al[:], ohu[:, :, :, 1:], czero4)
        PL.tensor_tensor(u1[:], u1[:], u2[:], op=OP.add)   # u1+u2 at chosen arm
        be = wq.tile([128, G, A, TCH], F32, tag="be")
        SC.activation(be[:], bcA(u2[:]), AF.Copy)
        V.copy_predicated(be[:], ohu[:, :, :, 1:], bcA(u1[:]))
        # chunk-carry: be[0] += al[0]*qprev ; al[0] = 0
        qp = qsprev[0] if c > 0 else bcT(pv[:, :, PRIOR], A)
        t1 = wp.tile([128, G, A], F32, tag="t1")
        V.tensor_tensor(t1[:], al[:, :, :, 0], qp, op=OP.mult)
        V.tensor_tensor(be[:, :, :, 0], be[:, :, :, 0], t1[:], op=OP.add)
        V.memset(al[:, :, :, 0:1], 0.0)

        # qs scan
        qs = wqs.tile([128, G, A, TCH], F32, tag="qs")
        V.tensor_tensor_scan(
            qs[:].rearrange("p g a t -> p (g a t)"),
            al[:].rearrange("p g a t -> p (g a t)"),
            be[:].rearrange("p g a t -> p (g a t)"),
            0.0, op0=OP.mult, op1=OP.add)
        qsprev[0] = qs[:, :, :, TCH - 1]

        # cum scan (+carry), ln1p
        cumH = wp.tile([128, G, A, TCH], F16, tag="cumH")
        V.tensor_tensor_scan(
            cumH[:].rearrange("p g a t -> p (g a t)"),
            rmH[:].rearrange("p g a t -> p (g a t)"),
            d["ohH"][:].rearrange("p g a t -> p (g a t)"),
            0.0, op0=OP.mult, op1=OP.add)
        cum = wq.tile([128, G, A, TCH], F32, tag="cum")
        PL.tensor_tensor(cum[:], cumH[:], bcT(cumc[:], TCH), op=OP.add)
        SC.activation(cumc[:], cum[:, :, :, TCH - 1], AF.Copy)
        SC.activation(cum[:], cum[:], AF.Ln, bias=c_one[:])   # ln(1+cum)

        # sm = A1*qs + beta_p*lncum
        sm = wq.tile([128, G, A, TCH], F32, tag="sm")
        for g in range(G):
            SC.activation(sm[:, g], qs[:, g], AF.Copy, scale=pv[:, g:g + 1, A1])
            SC.activation(cum[:, g], cum[:, g], AF.Copy, scale=pv[:, g:g + 1, BETA_P])
        PL.tensor_tensor(sm[:], sm[:], cum[:], op=OP.add)

        # softmax-log with pairwise A reductions
        mx2 = wp.tile([128, G, 2, TCH], F32, tag="mx2")
        V.tensor_tensor(mx2[:], sm[:, :, 0:2, :], sm[:, :, 2:4, :], op=OP.max)
        mx = wp.tile([128, G, TCH], F32, tag="mx")
        V.tensor_tensor(mx[:], mx2[:, :, 0, :], mx2[:, :, 1, :], op=OP.max)
        PL.tensor_tensor(sm[:], sm[:], bcA(mx[:]), op=OP.subtract)
        SC.activation(sm[:], sm[:], AF.Exp)
        V.tensor_tensor(mx2[:], sm[:, :, 0:2, :], sm[:, :, 2:4, :], op=OP.add)
        V.tensor_tensor(mx[:], mx2[:, :, 0, :], mx2[:, :, 1, :], op=OP.add)
        r = wp.tile([128, G, TCH], F32, tag="r")
        V.reciprocal_approx_fast(r[:], mx[:])
        for g in range(G):
            V.tensor_scalar(r[:, g], r[:, g], pv[:, g:g + 1, OML], None, op0=OP.mult)
            V.tensor_tensor(sm[:, g], sm[:, g],
                            r[:, g].unsqueeze(1).broadcast_to([128, A, TCH]), op=OP.mult)
            SC.activation(sm[:, g], sm[:, g], AF.Identity, bias=pv[:, g:g + 1, L4])
        SC.activation(sm[:], sm[:], AF.Ln)

        # tsl / same / inner
        same = wp.tile([128, G, TCH], F32, tag="same")
        V.tensor_tensor(same[:], cho[:, :, 1:], cho[:, :, 0:TCH], op=OP.is_equal)
        d0 = wp.tile([128, G, TCH], F32, tag="d0")
        V.tensor_tensor(d0[:], same[:], rmG, op=OP.mult)
        d1 = wp.tile([128, G, TCH], F32, tag="d1")
        SC.activation(d1[:], same[:], AF.Copy)
        V.tensor_scalar(tslc[:], tslc[:], 1.0, None, op0=OP.add)
        V.tensor_tensor(d1[:, :, 0], same[:, :, 0], tslc[:], op=OP.mult)
        tsl = wp.tile([128, G, TCH], F32, tag="tsl")
        V.tensor_tensor_scan(
            tsl[:].rearrange("p g t -> p (g t)"),
            d0[:].rearrange("p g t -> p (g t)"),
            d1[:].rearrange("p g t -> p (g t)"),
            0.0, op0=OP.mult, op1=OP.add)
        V.tensor_copy(tslc[:], tsl[:, :, TCH - 1])
        SC.activation(tsl[:], tsl[:], AF.Ln, bias=c_one[:])   # ln(1+tsl)
        inner = wp.tile([128, G, TCH], F32, tag="inner")
        PL.tensor_tensor(inner[:], same[:], bcT(pv[:, :, PWSW], TCH), op=OP.mult)
        PL.tensor_tensor(inner[:], inner[:], bcT(pv[:, :, SWX], TCH), op=OP.add)
        PL.tensor_tensor(inner[:], inner[:], tsl[:], op=OP.add)

        # cl += oh*inner + ab1*oh_old + ab2*oh_rot
        ext = wq.tile([128, G, A, TCH], F32, tag="ext")
        PL.tensor_tensor(ext[:], ohFc[:], bcA(inner[:]), op=OP.mult)
        exo = wq.tile([128, G, A, TCH], F32, tag="exo")
        for g in range(G):
            SC.activation(exo[:, g, :, 1:], ohFc[:, g, :, 0:TCH - 1],
                          AF.Copy, scale=pv[:, g:g + 1, AB1])
            SC.activation(exo[:, g, :, 0], ohu[:, g, :, 0],
                          AF.Copy, scale=pv[:, g:g + 1, AB1])
        PL.tensor_tensor(ext[:], ext[:], exo[:], op=OP.add)
        for g in range(G):
            SC.activation(exo[:, g, 0:2, :], ohFc[:, g, 2:4, :],
                          AF.Copy, scale=pv[:, g:g + 1, AB2])
            SC.activation(exo[:, g, 2:4, :], ohFc[:, g, 0:2, :],
                          AF.Copy, scale=pv[:, g:g + 1, AB2])
        PL.tensor_tensor(ext[:], ext[:], exo[:], op=OP.add)
        PL.tensor_tensor(sm[:], sm[:], ext[:], op=OP.add)
        nc.sync.dma_start(out_d[:, :, :, t0:t0 + TCH], sm[:])

    czt = pp.tile([128, 1], F32)
    V.memset(czt[:], 0.0)
    czero4 = czt[:].unsqueeze(2).unsqueeze(3).broadcast_to([128, G, A, TCH])

    # software pipeline: prep(c) -> loop(c) ; epilogue(c-1) interleaved
    prev = None
    dnext = prep(0)
    for c in range(NCH):
        d = dnext
        loop(c, d)
        if prev is not None:
            epilogue(c - 1, prev)
        if c + 1 < NCH:
            dnext = prep(c + 1)
        prev = d
    epilogue(NCH - 1, prev)


_NC_CACHE = [None]


def kernel(params, rewards, choices, pids):
    """Full-input host wrapper: shard B across 8 cores, run SPMD, gather."""
    if _NC_CACHE[0] is None:
        _NC_CACHE[0] = build_nc()
    nc = _NC_CACHE[0]
    paramsT = np.ascontiguousarray(params.T, dtype=np.float32)   # [P, 13]
    in_maps = []
    for k in range(NCORE):
        s0 = k * BL
        in_maps.append({
            "rewards": np.ascontiguousarray(rewards[:, s0:s0 + BL].T, np.float32),
            "choices": np.ascontiguousarray(choices[:, s0:s0 + BL].T, np.int32),
            "paramsT": paramsT,
            "pids": np.ascontiguousarray(pids[s0:s0 + BL].reshape(G, 128).T, np.int32),
        })
    res = run_bass_kernel_spmd(nc, in_maps, list(range(NCORE)), trace=False)
    out = np.empty((T, B, A), np.float32)
    for k in range(NCORE):
        o = res.results[k]["out"]          # [128, G, A, T]
        out[:, k * BL:(k + 1) * BL, :] = o.transpose(3, 1, 0, 2).reshape(T, BL, A)
    return out


# revision 23
# speedup vs baseline: 1.0222x; 1.0222x over previous
"""Trainium2 Bass kernel for the Castro2025 RL model (T=1000, B=8192, P=1024, A=4).

Sharding: batch (sessions) split across 8 NeuronCores; per-participant params
replicated; sequential scan over T runs on-device per core.

Structure per core (1024 sessions as [128 partitions, G=8 groups]):
 - scaled serial loop on Vector: 4 ops/trial emitting the per-trial sum
   sequence m-tilde; the per-step decay product A_t is folded into
   pre-divided targets within 10-step windows (rescale at window ends).
 - bulk reconstruction: per-arm affine scan (tensor_tensor_scan) gives
   q_t for all t; softmax/log pipeline with pairwise A-reductions,
   activations on the Scalar engine, mask work on GpSimd.
Self-contained: includes the harness patches it needs.
"""
import sys
import types
import numpy as np
import concourse.bass as bass
import concourse.bacc as bacc
import concourse.mybir as mybir
import concourse.tile as tile
from concourse.vector_clock import ScopedClock

# ---- harness patch: this walrus build rejects any instruction carrying more
# than one semaphore wait; split extras onto single-wait NOPs beforehand. ----
MAX_WAITS = 1
if not getattr(tile, "_waitsplit_patched", False):
    _orig_postorder = tile.postorder_instruction_blocks

    def _split_waits_postorder(ordered, start_bb, postordered):
        out = _orig_postorder(ordered, start_bb, postordered)
        for bb_name, insts in postordered.items():
            new_list = []
            for inst in insts:
                si = inst.sync_info
                if si is not None and si.on_wait and len(si.on_wait) > MAX_WAITS \
                        and inst.engine != mybir.EngineType.Unassigned \
                        and not isinstance(inst, tile.BassTileLoopBlock):
                    waits = list(si.on_wait)
                    keep = waits[-MAX_WAITS:]
                    extra = waits[:-MAX_WAITS]
                    for wi, w in enumerate(extra):
                        nop = mybir.InstNoOp(
                            name=f"I-waitsplit-{id(inst)}-{len(new_list)}-{wi}",
                            engine=inst.engine,
                            sync_info=mybir.SyncInfo(on_wait=[w], on_update=[]),
                        )
                        new_list.append(nop)
                    si.on_wait = keep
                new_list.append(inst)
            insts[:] = new_list
        return out

    tile.postorder_instruction_blocks = _split_waits_postorder

    def _patched_drain_and_barrier(self, tick_clock, wait_clock):
        probe = mybir.InstNoOp(name=f"I-{self.nc.next_id()}", engine=mybir.EngineType.SP)
        wait_clock.add_sem_waits(probe, ScopedClock({None: tick_clock.global_clock}))
        waits = list(probe.sync_info.on_wait) if probe.sync_info and probe.sync_info.on_wait else []
        for w in waits:
            nop = self.nc.sync.nop(nofuse=True, hint="drain_split_wait")
            if nop.ins.sync_info is None:
                nop.ins.sync_info = mybir.SyncInfo(on_wait=[w], on_update=[])
            else:
                nop.ins.sync_info.on_wait = [w]
        self.nc.sync.drain()
        self.nc.all_engine_barrier()
        popped = self.nc._tile_sem_poison_stack.pop()
        assert popped is self._sem_poison
        self.nc.clear_and_free_semaphores(list(self.sems.allocated().values()))
        self.nc.all_engine_barrier()

    tile.TileContext._drain_and_barrier = _patched_drain_and_barrier
    tile._waitsplit_patched = True

from concourse.bass_utils import run_bass_kernel_spmd  # noqa: E402

F32 = mybir.dt.float32
F16 = mybir.dt.float16
I32 = mybir.dt.int32
U8 = mybir.dt.uint8
AX = mybir.AxisListType
OP = mybir.AluOpType
AF = mybir.ActivationFunctionType

T, B, P, A = 1000, 8192, 1024, 4
NCORE = 8
BL = B // NCORE          # 1024 sessions per core
G = BL // 128            # 8 groups
ER_D = 1.0 - 1e-3
LN_ER_D = float(np.log(ER_D))
TCH = 50                 # chunk length
NCH = T // TCH
W = 10                   # scale window length
NW = TCH // W


def bcA(ap, n=A):
    """[...] -> broadcast a new axis before the last (trial) axis."""
    s = list(ap.shape)
    k = len(s) - 1
    return ap.unsqueeze(k).broadcast_to(s[:k] + [n] + s[k:])


def bcT(ap, n):
    """append broadcast trailing axis of n."""
    s = list(ap.shape)
    return ap.unsqueeze(len(s)).broadcast_to(s + [n])


def build_nc():
    nc = bacc.Bacc()
    rew_d = nc.declare_dram_parameter("rewards", [BL, T], F32, isOutput=False)
    cho_d = nc.declare_dram_parameter("choices", [BL, T], I32, isOutput=False)
    par_d = nc.declare_dram_parameter("paramsT", [P, 13], F32, isOutput=False)
    pid_d = nc.declare_dram_parameter("pids", [128, G], I32, isOutput=False)
    out_d = nc.declare_dram_parameter("out", [128, G, A, T], F32, isOutput=True)

    with tile.TileContext(nc) as tc:
        import contextlib
        with contextlib.ExitStack() as ctx:
            _build(ctx, tc, nc, rew_d, cho_d, par_d, pid_d, out_d)
    nc.compile()
    return nc


def _build(ctx, tc, nc, rew_d, cho_d, par_d, pid_d, out_d):
    pp = ctx.enter_context(tc.tile_pool(name="persist", bufs=1))
    wp = ctx.enter_context(tc.tile_pool(name="work", bufs=2))
    wq = ctx.enter_context(tc.tile_pool(name="workq", bufs=1))
    wqs = ctx.enter_context(tc.tile_pool(name="workqs", bufs=2))
    wpd = ctx.enter_context(tc.tile_pool(name="workd", bufs=3))

    V, PL, SC = nc.vector, nc.gpsimd, nc.scalar

    # ---------- params ----------
    pids = pp.tile([128, G], I32)
    nc.sync.dma_start(pids[:], pid_d[:])
    praw = pp.tile([128, G, 13], F32)
    for g in range(G):
        nc.gpsimd.indirect_dma_start(
            out=praw[:, g, :], out_offset=None, in_=par_d[:],
            in_offset=bass.IndirectOffsetOnAxis(ap=pids[:, g:g + 1], axis=0))
    V.tensor_scalar(praw[:], praw[:], -5.0, 5.0, op0=OP.max, op1=OP.min)

    c_one = pp.tile([128, 1], F32)
    V.memset(c_one[:], 1.0)
    c_lnd = pp.tile([128, 1], F32)
    V.memset(c_lnd[:], LN_ER_D)

    def sp(dst, src):
        SC.activation(dst, src, AF.Exp)
        SC.activation(dst, dst, AF.Ln, bias=c_one[:])

    def sg(dst, src):
        SC.activation(dst, src, AF.Sigmoid)

    def clip(ap, lo, hi):
        V.tensor_scalar(ap, ap, float(lo), float(hi), op0=OP.max, op1=OP.min)

    NPV = 20
    pv = pp.tile([128, G, NPV], F32)
    (BETA_R, LAPSE, PRIOR, AER, DECAY, AB1, AB2, PERV, SW, GAM, TEMP, BETA_P,
     A1, L4, OML, PWSW, GAMP1, QD, RD, SWX) = range(NPV)
    sp(pv[:, :, BETA_R], praw[:, :, 0]); clip(pv[:, :, BETA_R], 0.01, 20.0)
    sg(pv[:, :, LAPSE], praw[:, :, 1]); clip(pv[:, :, LAPSE], 0.01, 0.99)
    sp(pv[:, :, PRIOR], praw[:, :, 2]); clip(pv[:, :, PRIOR], 0.01, 0.99)
    sg(pv[:, :, AER], praw[:, :, 3]); clip(pv[:, :, AER], 0.01, 0.99)
    sg(pv[:, :, DECAY], praw[:, :, 4]); clip(pv[:, :, DECAY], 0.01, 0.99)
    V.tensor_copy(pv[:, :, AB1], praw[:, :, 5])
    V.tensor_copy(pv[:, :, AB2], praw[:, :, 6])
    sp(pv[:, :, PERV], praw[:, :, 7])
    V.tensor_copy(pv[:, :, SW], praw[:, :, 8])
    sp(pv[:, :, GAM], praw[:, :, 10])
    sp(pv[:, :, TEMP], praw[:, :, 11])
    V.tensor_scalar(pv[:, :, TEMP], pv[:, :, TEMP], 1e-6, None, op0=OP.add)
    clip(pv[:, :, TEMP], 1e-6, 100.0)
    sp(pv[:, :, BETA_P], praw[:, :, 12])
    rtmp = pp.tile([128, G], F32)
    V.reciprocal(rtmp[:], pv[:, :, TEMP])
    V.tensor_tensor(pv[:, :, A1], pv[:, :, BETA_R], rtmp[:], op=OP.mult)
    V.tensor_scalar(pv[:, :, L4], pv[:, :, LAPSE], 0.25, None, op0=OP.mult)
    V.tensor_scalar(pv[:, :, OML], pv[:, :, LAPSE], -1.0, 1.0, op0=OP.mult, op1=OP.add)
    V.tensor_tensor(pv[:, :, PWSW], pv[:, :, PERV], pv[:, :, SW], op=OP.subtract)
    V.tensor_scalar(pv[:, :, GAMP1], pv[:, :, GAM], 1.0, None, op0=OP.add)
    V.tensor_scalar(pv[:, :, QD], pv[:, :, DECAY], 0.25, None, op0=OP.mult)
    V.tensor_copy(pv[:, :, SWX], pv[:, :, SW])

    # dp[t] = 0.999^(t+1)
    dpow = pp.tile([128, T], F32)
    ii32 = pp.tile([128, T], I32)
    nc.gpsimd.iota(ii32[:], pattern=[[1, T]], base=0, channel_multiplier=0)
    V.tensor_copy(dpow[:], ii32[:])
    SC.activation(dpow[:], dpow[:], AF.Exp, bias=c_lnd[:], scale=c_lnd[:])

    # reset masks for per-chunk scans
    rmGA = pp.tile([128, G, A, TCH], F32)
    V.memset(rmGA[:], 1.0)
    V.memset(rmGA[:, :, :, 0:1], 0.0)
    rmG = rmGA[:, :, 0, :]
    rmH = pp.tile([128, G, A, TCH], F16)
    V.memset(rmH[:], 1.0)
    V.memset(rmH[:, :, :, 0:1], 0.0)


    # serial-loop state
    qT = pp.tile([128, G, A], F32)
    V.tensor_copy(qT[:], bcT(pv[:, :, PRIOR], A))
    uT = pp.tile([128, G], F32)

    # epilogue carries
    cumc = pp.tile([128, G, A], F32)
    V.memset(cumc[:], 0.0)
    tslc = pp.tile([128, G], F32)
    V.memset(tslc[:], 0.0)

    # ---------------- per-chunk prep (Pool + a couple of V ops) ----------------
    def prep_io(c):
        t0 = c * TCH
        cho = wpd.tile([128, G, TCH + 1], I32, tag="cho")
        if c == 0:
            V.memset(cho[:, :, 0:1], -1)
            nc.sync.dma_start(cho[:, :, 1:].opt(), cho_d[:, t0:t0 + TCH].rearrange("(g p) t -> p g t", p=128))
        else:
            nc.sync.dma_start(cho[:].opt(), cho_d[:, t0 - 1:t0 + TCH].rearrange("(g p) t -> p g t", p=128))
        rew = wpd.tile([128, G, TCH], F32, tag="rew")
        nc.sync.dma_start(rew[:], rew_d[:, t0:t0 + TCH].rearrange("(g p) t -> p g t", p=128))
        return {"cho": cho, "rew": rew}

    def prep(c, dio):
        t0 = c * TCH
        d = dict(dio)
        cho, rew = d["cho"], d["rew"]

        # one-hot masks: u8 (for copy_pred) and f32 (for scan d1 / mask mults)
        ohu = wp.tile([128, G, A, TCH + 1], U8, tag="ohu")
        ohFc = wp.tile([128, G, A, TCH], F32, tag="ohFc")
        for a in range(A):
            V.tensor_scalar(ohu[:, :, a, :], cho[:], float(a), None, op0=OP.is_equal)
        SC.activation(ohFc[:], ohu[:, :, :, 1:], AF.Copy)
        ohH = wp.tile([128, G, A, TCH], F16, tag="ohH")
        SC.activation(ohH[:], ohu[:, :, :, 1:], AF.Copy)

        # e = aer * dp_t ; At = decay*(1-e) ; w~ = e/(4(1-e)) ; Bt*m needs 0.25*decay*e
        dpc = dpow[:, t0:t0 + TCH].unsqueeze(1).broadcast_to([128, G, TCH])
        e = wp.tile([128, G, TCH], F32, tag="e")
        PL.tensor_tensor(e[:], bcT(pv[:, :, AER], TCH), dpc, op=OP.mult)
        ome = wp.tile([128, G, TCH], F32, tag="ome")
        PL.tensor_scalar(ome[:], e[:], -1.0, 1.0, op0=OP.mult, op1=OP.add)   # 1-e
        At = wp.tile([128, G, TCH], F32, tag="At")
        PL.tensor_tensor(At[:], bcT(pv[:, :, DECAY], TCH), ome[:], op=OP.mult)
        rome = wp.tile([128, G, TCH], F32, tag="rome")
        V.reciprocal_approx_fast(rome[:], ome[:])                         # 1/(1-e)
        wt = wp.tile([128, G, TCH], F32, tag="wt")
        PL.tensor_tensor(wt[:], e[:], rome[:], op=OP.mult)
        PL.tensor_scalar(wt[:], wt[:], 0.25, None, op0=OP.mult)

        # tgt = rew*(1+gam) - gam
        tgt = wp.tile([128, G, TCH], F32, tag="tgt")
        PL.tensor_tensor(tgt[:], rew[:], bcT(pv[:, :, GAMP1], TCH), op=OP.mult)
        PL.tensor_tensor(tgt[:], tgt[:], bcT(pv[:, :, GAM], TCH), op=OP.subtract)

        # windowed prefix products: Pw[w,j] = prod_{u<j} At[w,u]  (Pw[w,0]=1)
        # and rPw = 1/Pw (built from rAt = 1/At = rdecay * (1/(1-e)))
        Pw = wp.tile([128, G, NW, W], F32, tag="Pw")
        rPw = wp.tile([128, G, NW, W], F32, tag="rPw")
        rAt = wp.tile([128, G, TCH], F32, tag="rAt")
        PL.tensor_tensor(rAt[:], rome[:], bcT(rtmp2[:], TCH), op=OP.mult)
        if c < 2:
            PL.memset(Pw[:, :, :, 0:1], 1.0)
            PL.memset(rPw[:, :, :, 0:1], 1.0)
        Atw = At[:].rearrange("p g (w j) -> p g w j", j=W)
        rAtw = rAt[:].rearrange("p g (w j) -> p g w j", j=W)
        for j in range(1, W):
            PL.tensor_tensor(Pw[:, :, :, j], Pw[:, :, :, j - 1], Atw[:, :, :, j - 1], op=OP.mult)
            PL.tensor_tensor(rPw[:, :, :, j], rPw[:, :, :, j - 1], rAtw[:, :, :, j - 1], op=OP.mult)
        # Dwin[w] = Pw[w, 9] * At[w, 9]
        Dwin = wp.tile([128, G, NW], F32, tag="Dwin")
        PL.tensor_tensor(Dwin[:], Pw[:, :, :, W - 1], Atw[:, :, :, W - 1], op=OP.mult)
        # tau = tgt * rPw
        tau = wp.tile([128, G, TCH], F32, tag="tau")
        PL.tensor_tensor(tau[:], tgt[:], rPw[:].rearrange("p g w j -> p g (w j)"), op=OP.mult)

        d.update(ohu=ohu, ohFc=ohFc, ohH=ohH, e=e, At=At, wt=wt,
                 tgt=tgt, Pw=Pw, Dwin=Dwin, tau=tau)
        return d

    # 1/decay (needed for rAt) -- compute once
    rtmp2 = pp.tile([128, G], F32)
    V.reciprocal(rtmp2[:], pv[:, :, DECAY])

    # ---------------- serial loop over one chunk (Vector) ----------------
    def loop(c, d):
        mt = wp.tile([128, G, TCH], F32, tag="mt")
        d["mt"] = mt
        ohu, tau, wt, Dwin = d["ohu"], d["tau"], d["wt"], d["Dwin"]
        for ti in range(TCH):
            V.copy_predicated(qT[:], ohu[:, :, :, ti + 1],
                              bcT(tau[:, :, ti], A))
            V.tensor_reduce(mt[:, :, ti], qT[:], axis=AX.X, op=OP.add)
            V.tensor_tensor(uT[:], mt[:, :, ti], wt[:, :, ti], op=OP.mult)
            V.tensor_tensor(qT[:], qT[:], bcT(uT[:], A), op=OP.add)
            if ti % W == W - 1:
                V.tensor_tensor(qT[:], qT[:], bcT(Dwin[:, :, ti // W], A), op=OP.mult)

    # ---------------- epilogue (bulk reconstruction; Pool/Act/V) ----------------
    qsprev = [None]

    def epilogue(c, d):
        t0 = c * TCH
        ohu, ohFc = d["ohu"], d["ohFc"]
        e, At, tgt, Pw, tau = d["e"], d["At"], d["tgt"], d["Pw"], d["tau"]
        cho, rew = d["cho"], d["rew"]
        At4 = bcA(At[:])

        # m = m~ * Pw ; u1 = At*tgt ; u2 = 0.25*decay*e*m
        m = wp.tile([128, G, TCH], F32, tag="m")
        PL.tensor_tensor(m[:], d["mt"][:], Pw[:].rearrange("p g w j -> p g (w j)"), op=OP.mult)
        u1 = wp.tile([128, G, TCH], F32, tag="u1")
        PL.tensor_tensor(u1[:], At[:], tgt[:], op=OP.mult)
        u2 = wp.tile([128, G, TCH], F32, tag="u2")
        PL.tensor_tensor(u2[:], e[:], m[:], op=OP.mult)
        PL.tensor_tensor(u2[:], u2[:], bcT(pv[:, :, QD], TCH), op=OP.mult)

        # al = At*(1-oh) ; be = oh*u1 + u2   (copy + copy_pred injection)
        al = wq.tile([128, G, A, TCH], F32, tag="al")
        SC.activation(al[:], At4, AF.Copy)
        V.copy_predicated(al[:], ohu[:, :, :, 1:], czero4)
        PL.tensor_tensor(u1[:], u1[:], u2[:], op=OP.add)   # u1+u2 at chosen arm
        be = wq.tile([128, G, A, TCH], F32, tag="be")
        SC.activation(be[:], bcA(u2[:]), AF.Copy)
        V.copy_predicated(be[:], ohu[:, :, :, 1:], bcA(u1[:]))
        # chunk-carry: be[0] += al[0]*qprev ; al[0] = 0
        qp = qsprev[0] if c > 0 else bcT(pv[:, :, PRIOR], A)
        t1 = wp.tile([128, G, A], F32, tag="t1")
        V.tensor_tensor(t1[:], al[:, :, :, 0], qp, op=OP.mult)
        V.tensor_tensor(be[:, :, :, 0], be[:, :, :, 0], t1[:], op=OP.add)
        V.memset(al[:, :, :, 0:1], 0.0)

        # qs scan
        qs = wqs.tile([128, G, A, TCH], F32, tag="qs")
        V.tensor_tensor_scan(
            qs[:].rearrange("p g a t -> p (g a t)"),
            al[:].rearrange("p g a t -> p (g a t)"),
            be[:].rearrange("p g a t -> p (g a t)"),
            0.0, op0=OP.mult, op1=OP.add)
        qsprev[0] = qs[:, :, :, TCH - 1]

        # cum scan (+carry), ln1p
        cumH = wp.tile([128, G, A, TCH], F16, tag="cumH")
        V.tensor_tensor_scan(
            cumH[:].rearrange("p g a t -> p (g a t)"),
            rmH[:].rearrange("p g a t -> p (g a t)"),
            d["ohH"][:].rearrange("p g a t -> p (g a t)"),
            0.0, op0=OP.mult, op1=OP.add)
        cum = wq.tile([128, G, A, TCH], F32, tag="cum")
        V.tensor_tensor(cum[:], cumH[:], bcT(cumc[:], TCH), op=OP.add)
        SC.activation(cumc[:], cum[:, :, :, TCH - 1], AF.Copy)
        SC.activation(cum[:], cum[:], AF.Ln, bias=c_one[:])   # ln(1+cum)

        # sm = A1*qs + beta_p*lncum
        sm = wq.tile([128, G, A, TCH], F32, tag="sm")
        for g in range(G):
            SC.activation(sm[:, g], qs[:, g], AF.Copy, scale=pv[:, g:g + 1, A1])
            SC.activation(cum[:, g], cum[:, g], AF.Copy, scale=pv[:, g:g + 1, BETA_P])
        PL.tensor_tensor(sm[:], sm[:], cum[:], op=OP.add)

        # softmax-log with pairwise A reductions
        mx2 = wp.tile([128, G, 2, TCH], F32, tag="mx2")
        V.tensor_tensor(mx2[:], sm[:, :, 0:2, :], sm[:, :, 2:4, :], op=OP.max)
        mx = wp.tile([128, G, TCH], F32, tag="mx")
        V.tensor_tensor(mx[:], mx2[:, :, 0, :], mx2[:, :, 1, :], op=OP.max)
        V.tensor_tensor(sm[:], sm[:], bcA(mx[:]), op=OP.subtract)
        SC.activation(sm[:], sm[:], AF.Exp)
        V.tensor_tensor(mx2[:], sm[:, :, 0:2, :], sm[:, :, 2:4, :], op=OP.add)
        V.tensor_tensor(mx[:], mx2[:, :, 0, :], mx2[:, :, 1, :], op=OP.add)
        r = wp.tile([128, G, TCH], F32, tag="r")
        V.reciprocal_approx_fast(r[:], mx[:])
        for g in range(G):
            V.tensor_scalar(r[:, g], r[:, g], pv[:, g:g + 1, OML], None, op0=OP.mult)
            V.tensor_tensor(sm[:, g], sm[:, g],
                            r[:, g].unsqueeze(1).broadcast_to([128, A, TCH]), op=OP.mult)
            SC.activation(sm[:, g], sm[:, g], AF.Identity, bias=pv[:, g:g + 1, L4])
        SC.activation(sm[:], sm[:], AF.Ln)

        # tsl / same / inner
        same = wp.tile([128, G, TCH], F32, tag="same")
        V.tensor_tensor(same[:], cho[:, :, 1:], cho[:, :, 0:TCH], op=OP.is_equal)
        d0 = wp.tile([128, G, TCH], F32, tag="d0")
        V.tensor_tensor(d0[:], same[:], rmG, op=OP.mult)
        d1 = wp.tile([128, G, TCH], F32, tag="d1")
        SC.activation(d1[:], same[:], AF.Copy)
        V.tensor_scalar(tslc[:], tslc[:], 1.0, None, op0=OP.add)
        V.tensor_tensor(d1[:, :, 0], same[:, :, 0], tslc[:], op=OP.mult)
        tsl = wp.tile([128, G, TCH], F32, tag="tsl")
        V.tensor_tensor_scan(
            tsl[:].rearrange("p g t -> p (g t)"),
            d0[:].rearrange("p g t -> p (g t)"),
            d1[:].rearrange("p g t -> p (g t)"),
            0.0, op0=OP.mult, op1=OP.add)
        V.tensor_copy(tslc[:], tsl[:, :, TCH - 1])
        SC.activation(tsl[:], tsl[:], AF.Ln, bias=c_one[:])   # ln(1+tsl)
        inner = wp.tile([128, G, TCH], F32, tag="inner")
        PL.tensor_tensor(inner[:], same[:], bcT(pv[:, :, PWSW], TCH), op=OP.mult)
        PL.tensor_tensor(inner[:], inner[:], bcT(pv[:, :, SWX], TCH), op=OP.add)
        PL.tensor_tensor(inner[:], inner[:], tsl[:], op=OP.add)

        # cl += oh*inner + ab1*oh_old + ab2*oh_rot
        ext = wq.tile([128, G, A, TCH], F32, tag="ext")
        PL.tensor_tensor(ext[:], ohFc[:], bcA(inner[:]), op=OP.mult)
        exo = wq.tile([128, G, A, TCH], F32, tag="exo")
        for g in range(G):
            SC.activation(exo[:, g, :, 1:], ohFc[:, g, :, 0:TCH - 1],
                          AF.Copy, scale=pv[:, g:g + 1, AB1])
            SC.activation(exo[:, g, :, 0], ohu[:, g, :, 0],
                          AF.Copy, scale=pv[:, g:g + 1, AB1])
        PL.tensor_tensor(ext[:], ext[:], exo[:], op=OP.add)
        for g in range(G):
            SC.activation(exo[:, g, 0:2, :], ohFc[:, g, 2:4, :],
                          AF.Copy, scale=pv[:, g:g + 1, AB2])
            SC.activation(exo[:, g, 2:4, :], ohFc[:, g, 0:2, :],
                          AF.Copy, scale=pv[:, g:g + 1, AB2])
        PL.tensor_tensor(ext[:], ext[:], exo[:], op=OP.add)
        PL.tensor_tensor(sm[:], sm[:], ext[:], op=OP.add)
        nc.sync.dma_start(out_d[:, :, :, t0:t0 + TCH], sm[:])

    czt = pp.tile([128, 1], F32)
    V.memset(czt[:], 0.0)
    czero4 = czt[:].unsqueeze(2).unsqueeze(3).broadcast_to([128, G, A, TCH])

    # software pipeline: IO DMAs prefetch 2 chunks ahead; rest of prep 1 ahead
    ios = {0: prep_io(0), 1: prep_io(1)}
    prev = None
    dnext = prep(0, ios.pop(0))
    for c in range(NCH):
        d = dnext
        loop(c, d)
        if prev is not None:
            epilogue(c - 1, prev)
        if c + 2 < NCH:
            ios[c + 2] = prep_io(c + 2)
        if c + 1 < NCH:
            dnext = prep(c + 1, ios.pop(c + 1))
        prev = d
    epilogue(NCH - 1, prev)


_NC_CACHE = [None]


def kernel(params, rewards, choices, pids):
    """Full-input host wrapper: shard B across 8 cores, run SPMD, gather."""
    if _NC_CACHE[0] is None:
        _NC_CACHE[0] = build_nc()
    nc = _NC_CACHE[0]
    paramsT = np.ascontiguousarray(params.T, dtype=np.float32)   # [P, 13]
    in_maps = []
    for k in range(NCORE):
        s0 = k * BL
        in_maps.append({
            "rewards": np.ascontiguousarray(rewards[:, s0:s0 + BL].T, np.float32),
            "choices": np.ascontiguousarray(choices[:, s0:s0 + BL].T, np.int32),
            "paramsT": paramsT,
            "pids": np.ascontiguousarray(pids[s0:s0 + BL].reshape(G, 128).T, np.int32),
        })
    res = run_bass_kernel_spmd(nc, in_maps, list(range(NCORE)), trace=False)
    out = np.empty((T, B, A), np.float32)
    for k in range(NCORE):
        o = res.results[k]["out"]          # [128, G, A, T]
        out[:, k * BL:(k + 1) * BL, :] = o.transpose(3, 1, 0, 2).reshape(T, BL, A)
    return out
